# revision 5
# baseline (speedup 1.0000x reference)
"""Causal multi-head attention with RoPE for TRN2 (Bass/Tile), 8 NeuronCores.

Problem: y = (softmax(causal(rope(x@Wq) @ rope(x@Wk)^T / sqrt(dh))) @ (x@Wv)) @ Wo
  B=4, T=2048, D=2048, H=16 heads, dh=128, fp32 I/O.

Sharding: 4-way batch data-parallel x 2-way head tensor-parallel.
  Core c: batch b = c//2, head group g = c%2 (heads 8g..8g+7).
  Each core computes a partial y[b] (its 8 heads' contribution through Wo);
  the host sums the two partials per batch.

All matmuls run in fp16 (full PE rate; fp32 PSUM accumulation). Everything is
kept transposed so no on-chip transposes are needed:
  - Projections contract over D with x^T resident in SBUF: Q^T/K^T produced as
    [dh, t]; V as [t, dh].
  - S^T[k, q] = (K^T chunk)^T-matmul with Q^T as the moving operand.
  - exp(S^T) is directly the lhsT of the P@V matmul -> O^T [dh, q].
  - O^T is directly the lhsT of the Wo projection.
Softmax: no max subtraction (logits are O(+-6), exp is fp32-safe); the
denominator comes from a ones-vector matmul accumulated alongside P@V.
RoPE: pair partners are pre-permuted into partition halves (even dh dims ->
partitions 0..63, odd -> 64..127) via a host-side column permutation of Wq/Wk,
making rotate-half a uniform +-64-partition shift on chip.
"""

import numpy as np

import concourse.bass as bass
import concourse.tile as tile
from concourse import bacc, mybir
from concourse.bass import ts
from concourse.bass_utils import run_bass_kernel_spmd

B, T, D = 4, 2048, 2048
H = 16
DH = 128
THETA = 10000.0
NCORES = 8
HPC = H // 2  # heads per core (2-way head TP)
P = 128
TQ = 512  # q-tile width
F16 = mybir.dt.float16
F32 = mybir.dt.float32


def build(t=T, d=D, hpc=HPC):
    """Build the per-core Bass program (same program on all cores)."""
    nc = bacc.Bacc("TRN2", target_bir_lowering=False, debug=False)
    dc = d // P  # contraction chunks for projections
    tc_n = t // P  # token chunks (k-chunks in attention)
    ntq = t // TQ  # q tiles
    vg = max(1, hpc // 4)  # V head-groups of up to 4 heads (N=512)
    vgh = hpc // vg  # heads per V group
    vgw = vgh * DH  # V group width

    xt = nc.dram_tensor("xt", [d, t], F16, kind="ExternalInput").ap()
    wq = nc.dram_tensor("wq", [hpc, d, DH], F16, kind="ExternalInput").ap()
    wk = nc.dram_tensor("wk", [hpc, d, DH], F16, kind="ExternalInput").ap()
    wv = nc.dram_tensor("wv", [vg, d, vgw], F16, kind="ExternalInput").ap()
    wo = nc.dram_tensor("wo", [hpc * DH, d], F16, kind="ExternalInput").ap()
    cos = nc.dram_tensor("cos", [P, t], F32, kind="ExternalInput").ap()
    sin = nc.dram_tensor("sin", [P, t], F32, kind="ExternalInput").ap()
    maskm = nc.dram_tensor("maskm", [P, TQ // P, TQ], F16, kind="ExternalInput").ap()
    ones = nc.dram_tensor("ones", [P, P], F16, kind="ExternalInput").ap()
    y = nc.dram_tensor("y", [t, d], F32, kind="ExternalOutput").ap()

    with tile.TileContext(nc) as tc:
        with (
            tc.tile_pool(name="const", bufs=1) as constp,
            tc.tile_pool(name="aot", bufs=1) as aotp,
            tc.tile_pool(name="dram", bufs=1, space="DRAM") as dramp,
        ):
            cos_sb = constp.tile([P, t], F32, tag="cos")
            sin_sb = constp.tile([P, t], F32, tag="sin")
            mask_sb = constp.tile([P, TQ // P, TQ], F16, tag="mask")
            ones_sb = constp.tile([P, P], F16, tag="ones")
            nc.sync.dma_start(cos_sb[:], cos)
            nc.sync.dma_start(sin_sb[:], sin)
            nc.sync.dma_start(mask_sb[:], maskm)
            nc.sync.dma_start(ones_sb[:], ones)

            qt_dram = dramp.tile([hpc, P, t], F16, tag="qt")
            kt_dram = dramp.tile([hpc, P, t], F16, tag="kt")
            v_dram = dramp.tile([vg, t, vgw], F16, tag="v")

            # ---------------- Stage A: QKV projections + RoPE ----------------
            with (
                tc.tile_pool(name="xt", bufs=1) as xtp,
                tc.tile_pool(name="wstream", bufs=2) as wsp,
                tc.tile_pool(name="adrain", bufs=3) as adp,
                tc.tile_pool(name="apsum", bufs=2, space="PSUM") as apsp,
            ):
                xt_sb = xtp.tile([P, dc, t], F16, tag="xt")
                nc.sync.dma_start(xt_sb[:], xt.rearrange("(c p) t -> p c t", p=P))

                # V: head groups of vgh, moving operand = Wv (N = vgw)
                for g in range(vg):
                    wv_sb = wsp.tile([P, dc, vgw], F16, tag="wv")
                    nc.sync.dma_start(
                        wv_sb[:], wv[g].rearrange("(c p) m -> p c m", p=P)
                    )
                    for tt in range(tc_n):
                        pv = apsp.tile([P, vgw], F32, tag="pv")
                        for c in range(dc):
                            nc.tensor.matmul(
                                pv[:],
                                xt_sb[:, c, ts(tt, P)],
                                wv_sb[:, c, :],
                                start=(c == 0),
                                stop=(c == dc - 1),
                            )
                        vtile = adp.tile([P, vgw], F16, tag="vtile")
                        nc.vector.tensor_copy(vtile[:], pv[:])
                        nc.sync.dma_start(v_dram[g, ts(tt, P), :], vtile[:])

                # Q^T and K^T with fused RoPE
                for h in range(hpc):
                    for name, w_ap, out_dram in (
                        ("q", wq[h], qt_dram),
                        ("k", wk[h], kt_dram),
                    ):
                        w_sb = wsp.tile([P, dc, DH], F16, tag=f"w{name}")
                        nc.sync.dma_start(
                            w_sb[:], w_ap.rearrange("(c p) m -> p c m", p=P)
                        )
                        for jt in range(t // TQ):
                            pq = apsp.tile([P, TQ], F32, tag="pqk")
                            for c in range(dc):
                                nc.tensor.matmul(
                                    pq[:],
                                    w_sb[:, c, :],
                                    xt_sb[:, c, ts(jt, TQ)],
                                    start=(c == 0),
                                    stop=(c == dc - 1),
                                )
                            # rope: out = pq*cos + rot(pq)*sin, with
                            # rot[0:64] = -pq[64:128], rot[64:128] = pq[0:64]
                            rot = adp.tile([P, TQ], F32, tag="rot")
                            nc.scalar.mul(rot[0:64, :], pq[64:128, :], -1.0)
                            nc.scalar.copy(rot[64:128, :], pq[0:64, :])
                            t1 = adp.tile([P, TQ], F32, tag="t1")
                            nc.vector.tensor_mul(t1[:], pq[:], cos_sb[:, ts(jt, TQ)])
                            nc.vector.tensor_mul(rot[:], rot[:], sin_sb[:, ts(jt, TQ)])
                            qk_tile = adp.tile([P, TQ], F16, tag="qktile")
                            nc.vector.tensor_add(qk_tile[:], t1[:], rot[:])
                            nc.sync.dma_start(out_dram[h, :, ts(jt, TQ)], qk_tile[:])

            # ---------------- Stage B: attention per head ----------------
            aot_sb = aotp.tile([P, hpc, t], F16, tag="aot")
            with (
                tc.tile_pool(name="bstream", bufs=2) as bsp,
                tc.tile_pool(name="exps", bufs=6) as esp,
                tc.tile_pool(name="small", bufs=4) as smp,
                tc.tile_pool(name="bpsum", bufs=2, space="PSUM") as bpsp,
                tc.tile_pool(name="spsum", bufs=3, space="PSUM") as spsp,
            ):
                for g in range(vg):
                    v_sb = bsp.tile([P, tc_n, vgw], F16, tag="v")
                    nc.sync.dma_start(
                        v_sb[:], v_dram[g].rearrange("(c p) m -> p c m", p=P)
                    )
                    for hh in range(vgh):
                        h = g * vgh + hh
                        qt_sb = bsp.tile([P, t], F16, tag="qt")
                        kt_sb = bsp.tile([P, t], F16, tag="kt")
                        nc.sync.dma_start(qt_sb[:], qt_dram[h])
                        nc.sync.dma_start(kt_sb[:], kt_dram[h])
                        for jt in range(ntq):
                            op = bpsp.tile([P, TQ], F32, tag="op")
                            rp = bpsp.tile([P, TQ], F32, tag="rp")
                            nch = (jt + 1) * (TQ // P)
                            for c in range(nch):
                                sp = spsp.tile([P, TQ], F32, tag="sp")
                                nc.tensor.matmul(
                                    sp[:],
                                    kt_sb[:, ts(c, P)],
                                    qt_sb[:, ts(jt, TQ)],
                                    start=True,
                                    stop=True,
                                )
                                es = esp.tile([P, TQ], F16, tag="es")
                                o = c - jt * (TQ // P)
                                if o >= 0:
                                    # diagonal chunk: mask after exp
                                    tmp = esp.tile([P, TQ], F16, tag="estmp")
                                    nc.scalar.activation(
                                        tmp[:], sp[:], mybir.ActivationFunctionType.Exp
                                    )
                                    nc.vector.tensor_mul(
                                        es[:], tmp[:], mask_sb[:, o, :]
                                    )
                                else:
                                    nc.scalar.activation(
                                        es[:], sp[:], mybir.ActivationFunctionType.Exp
                                    )
                                nc.tensor.matmul(
                                    op[:],
                                    v_sb[:, c, ts(hh, DH)],
                                    es[:],
                                    start=(c == 0),
                                    stop=(c == nch - 1),
                                )
                                nc.tensor.matmul(
                                    rp[:],
                                    ones_sb[:],
                                    es[:],
                                    start=(c == 0),
                                    stop=(c == nch - 1),
                                )
                            rs = smp.tile([P, TQ], F32, tag="rs")
                            nc.vector.reciprocal(rs[:], rp[:])
                            nc.vector.tensor_mul(
                                aot_sb[:, h, ts(jt, TQ)], op[:], rs[:]
                            )

            # ---------------- Stage C: output projection ----------------
            with (
                tc.tile_pool(name="wo", bufs=1) as wop,
                tc.tile_pool(name="cdrain", bufs=3) as cdp,
                tc.tile_pool(name="cpsum", bufs=2, space="PSUM") as cpsp,
            ):
                wo_sb = wop.tile([P, hpc, d], F16, tag="wo")
                nc.sync.dma_start(wo_sb[:], wo.rearrange("(h p) n -> p h n", p=P))
                for tt in range(tc_n):
                    for nt in range(d // TQ):
                        yp = cpsp.tile([P, TQ], F32, tag="yp")
                        for h in range(hpc):
                            nc.tensor.matmul(
                                yp[:],
                                aot_sb[:, h, ts(tt, P)],
                                wo_sb[:, h, ts(nt, TQ)],
                                start=(h == 0),
                                stop=(h == hpc - 1),
                            )
                        ytile = cdp.tile([P, TQ], F32, tag="ytile")
                        nc.vector.tensor_copy(ytile[:], yp[:])
                        nc.sync.dma_start(y[ts(tt, P), ts(nt, TQ)], ytile[:])

    nc.compile()
    return nc


def _rope_tables(t=T):
    """cos/sin in transposed+permuted layout [128, t].

    Partition p < 64 holds dh dim 2p (even), p >= 64 holds dh dim 2(p-64)+1;
    pair (2i, 2i+1) shares inv_freq[i], so row p uses inv_freq[p % 64].
    """
    inv_freq = 1.0 / (THETA ** (np.arange(0, DH, 2, dtype=np.float64) / DH))  # [64]
    pos = np.arange(t, dtype=np.float64)
    freqs = pos[None, :] * inv_freq[np.arange(P) % 64][:, None]  # [128, t]
    return (
        np.cos(freqs).astype(np.float32),
        np.sin(freqs).astype(np.float32),
    )


def _perm():
    """Within-head dh permutation: even dims first, then odd dims."""
    return np.concatenate([np.arange(0, DH, 2), np.arange(1, DH, 2)])


def _masks(tq=TQ):
    """maskm[dk, o, dq] = 1 if dk <= dq - 128*o else 0 (diagonal-chunk masks)."""
    dk = np.arange(P)[:, None, None]
    o = np.arange(tq // P)[None, :, None]
    dq = np.arange(tq)[None, None, :]
    return (dk <= dq - P * o).astype(np.float16)


def prep_core_inputs(x_b, Wq_g, Wk_g, Wv_g, Wo_g, t=T, hpc=HPC):
    """Host-side input prep for one core.

    x_b: [t, D] (this core's batch); W*_g: this core's head-group slices
    (Wq/Wk/Wv: [D, hpc*DH] columns, Wo: [hpc*DH, D] rows).
    """
    d = x_b.shape[1]
    perm = _perm()
    scale = 1.0 / np.sqrt(DH)
    vg = max(1, hpc // 4)
    vgw = (hpc // vg) * DH

    wq = np.empty((hpc, d, DH), np.float16)
    wk = np.empty((hpc, d, DH), np.float16)
    for h in range(hpc):
        blk_q = Wq_g[:, h * DH : (h + 1) * DH]
        blk_k = Wk_g[:, h * DH : (h + 1) * DH]
        wq[h] = (blk_q[:, perm] * scale).astype(np.float16)
        wk[h] = blk_k[:, perm].astype(np.float16)

    cos, sin = _rope_tables(t)
    return {
        "xt": np.ascontiguousarray(x_b.T).astype(np.float16),
        "wq": wq,
        "wk": wk,
        "wv": np.ascontiguousarray(
            Wv_g.astype(np.float16).reshape(d, vg, vgw).transpose(1, 0, 2)
        ),
        "wo": Wo_g.astype(np.float16),
        "cos": cos,
        "sin": sin,
        "maskm": _masks(),
        "ones": np.ones((P, P), np.float16),
    }


def make_in_maps(inputs):
    x, Wq, Wk, Wv, Wo = (
        np.asarray(inputs["x"]),
        np.asarray(inputs["Wq"]),
        np.asarray(inputs["Wk"]),
        np.asarray(inputs["Wv"]),
        np.asarray(inputs["Wo"]),
    )
    in_maps = []
    for c in range(NCORES):
        b, g = c // 2, c % 2
        cols = slice(g * HPC * DH, (g + 1) * HPC * DH)
        in_maps.append(
            prep_core_inputs(x[b], Wq[:, cols], Wk[:, cols], Wv[:, cols], Wo[cols, :])
        )
    return in_maps


def _build_sharded(nc, n_cores=NCORES):
    """Build a reusable jitted 8-core executable (bass2jax multi-core path,
    without output donation so it can be re-invoked for timing)."""
    import jax
    from jax.experimental.shard_map import shard_map
    from jax.sharding import Mesh, NamedSharding, PartitionSpec

    from concourse import bass2jax

    bass2jax.install_neuronx_cc_hook()
    partition_name = nc.partition_id_tensor.name if nc.partition_id_tensor else None
    in_names, out_names, out_avals, zero_outs = [], [], [], []
    for alloc in nc.m.functions[0].allocations:
        if not isinstance(alloc, mybir.MemoryLocationSet):
            continue
        name = alloc.memorylocations[0].name
        if alloc.kind == "ExternalInput":
            if name != partition_name:
                in_names.append(name)
        elif alloc.kind == "ExternalOutput":
            out_names.append(name)
            shape = tuple(alloc.tensor_shape)
            dtype = mybir.dt.np(alloc.dtype)
            out_avals.append(jax.core.ShapedArray(shape, dtype))
            zero_outs.append(np.zeros(shape, dtype))
    n_params = len(in_names)
    all_names = in_names + out_names
    if partition_name is not None:
        all_names = all_names + [partition_name]

    def _body(*args):
        operands = list(args)
        if partition_name is not None:
            operands.append(bass2jax.partition_id_tensor())
        outs = bass2jax._bass_exec_p.bind(
            *operands,
            out_avals=tuple(out_avals),
            in_names=tuple(all_names),
            out_names=tuple(out_names),
            lowering_input_output_aliases=(),
            sim_require_finite=True,
            sim_require_nnan=True,
            nc=nc,
        )
        return tuple(outs)

    devices = jax.devices()[:n_cores]
    mesh = Mesh(np.asarray(devices), ("core",))
    in_specs = (PartitionSpec("core"),) * (n_params + len(out_names))
    out_specs = (PartitionSpec("core"),) * len(out_names)
    fn = jax.jit(
        shard_map(
            _body, mesh=mesh, in_specs=in_specs, out_specs=out_specs, check_rep=False
        ),
        keep_unused=True,
    )
    sharding = NamedSharding(mesh, PartitionSpec("core"))
    return fn, sharding, in_names, out_names, out_avals, zero_outs


def run_timed(nc, in_maps, reps=8, n_cores=NCORES):
    """Run on all cores; return (per-core results, min wall-clock exec ns)."""
    import time

    import jax

    fn, sharding, in_names, out_names, out_avals, zero_outs = _build_sharded(
        nc, n_cores
    )
    concat_in = [
        np.concatenate([np.asarray(in_maps[c][n]) for c in range(n_cores)], axis=0)
        for n in in_names
    ]
    concat_zeros = [
        np.zeros((n_cores * z.shape[0], *z.shape[1:]), z.dtype) for z in zero_outs
    ]
    dev_in = [jax.device_put(a, sharding) for a in concat_in]
    dev_zeros = [jax.device_put(a, sharding) for a in concat_zeros]
    out = jax.block_until_ready(fn(*dev_in, *dev_zeros))
    times = []
    for _ in range(reps):
        t0 = time.perf_counter()
        out = jax.block_until_ready(fn(*dev_in, *dev_zeros))
        times.append(time.perf_counter() - t0)
    exec_ns = int(min(times) * 1e9) if times else None
    results = [
        {
            name: np.asarray(out[i]).reshape(n_cores, *out_avals[i].shape)[c]
            for i, name in enumerate(out_names)
        }
        for c in range(n_cores)
    ]
    return results, exec_ns


def kernel(x, Wq, Wk, Wv, Wo):
    nc = build()
    in_maps = make_in_maps({"x": x, "Wq": Wq, "Wk": Wk, "Wv": Wv, "Wo": Wo})
    results = run_bass_kernel_spmd(nc, in_maps, core_ids=list(range(NCORES))).results
    out = np.empty((B, T, D), np.float32)
    for b in range(B):
        out[b] = results[2 * b]["y"] + results[2 * b + 1]["y"]
    return out
